# revision 1
# baseline (speedup 1.0000x reference)
"""Multi-head attention (12 heads, RoPE, causal SDPA) for Trainium2, 8 cores.

Sharding: batch (2) x head-group (4 groups of 3 heads). Each core computes,
for its (batch b, head-group hg): QKV projection for its 3 heads, RoPE,
causal attention, and a partial out-projection [T, C] restricted to its
heads' rows of w_out. The host sums the 4 head-group partials per batch.

Device-side layouts (T = 2048, C = 768, D = 64 per head):
  xT   [768, 2048]  x[b] transposed (c on partitions)
  wA   [768, 640]   packed lhsT weights: cols 0:128 [q0|q1], 128:256 [k0|k1],
                    256:320 q2, 320:384 k2, 384:576 w_v (3 heads), 64 zero pad
  wo   [64, 2304]   w_out rows for this head-group: 3 x [64 d, 768 c]
  cosT/sinT [128, 2048]  RoPE tables transposed, stacked twice (64 d x 2)
  rT   [128, 128]   rotate_half as matmul lhsT: rot(q)T_chunk = rT.T @ qT_chunk
  tri  [128, 128]   tri[kr, qc] = 1 if qc >= kr (causal keep-mask, S^T layout)

Attention is computed transposed (S^T[k, q] = K Q^T blocks) so that softmax
P^T lands in [k, q] layout, which feeds P@V directly with v in natural [t, d]
layout (no transposes). Softmax has no max-subtraction (scores are O(1) by
construction) and the denominator comes from an all-ones column appended to
the stationary v operand. Normalization is applied in the [d, q] layout via a
K=1 outer-product broadcast of 1/denominator. Matmuls use float32r (~13
mantissa bits, 4x faster than fp32 on the PE).
"""
import numpy as np

B, T, C, H, D = 2, 2048, 768, 12, 64
HPG = 3                    # heads per group
NG = B * (H // HPG)        # 8 cores
ROPE_BASE = 10000.0
TQ = T // 128              # 16 t-tiles
NCC = C // 128             # 6 contraction chunks
GW = 1024                  # attention q-group width
NGRP = T // GW             # 2 q-groups

_CACHE = {}


def _build_nc(reps=1):
    from concourse import bacc, tile, mybir

    f32 = mybir.dt.float32
    f32r = mybir.dt.float32r
    Exp = mybir.ActivationFunctionType.Exp
    mult = mybir.AluOpType.mult
    add = mybir.AluOpType.add

    nc = bacc.Bacc("TRN2", target_bir_lowering=False, debug=False,
                   num_devices=NG)

    xT_d = nc.dram_tensor("xT", [C, T], f32r, kind="ExternalInput").ap()
    wA_d = nc.dram_tensor("wA", [C, 640], f32r, kind="ExternalInput").ap()
    woA_d = nc.dram_tensor("woA", [2 * D, C], f32r, kind="ExternalInput").ap()
    woB_d = nc.dram_tensor("woB", [D, C], f32r, kind="ExternalInput").ap()
    cosT_d = nc.dram_tensor("cosT", [128, T], f32, kind="ExternalInput").ap()
    sinT_d = nc.dram_tensor("sinT", [128, T], f32, kind="ExternalInput").ap()
    rT_d = nc.dram_tensor("rT", [128, 128], f32r, kind="ExternalInput").ap()
    tri_d = nc.dram_tensor("tri", [128, 128], f32, kind="ExternalInput").ap()
    out_d = nc.dram_tensor("out", [T, C], f32, kind="ExternalOutput").ap()

    with tile.TileContext(nc) as tc:
      for rep in range(reps):
        with tc.tile_pool(name=f"persist{rep}", bufs=1) as pp:
                dmaq = [nc.sync, nc.scalar, nc.gpsimd]

                # ---- persistent constants ----
                woA = pp.tile([2 * D, C], f32r, tag="woA")
                dmaq[1].dma_start(woA[:], woA_d[:])
                woB = pp.tile([D, C], f32r, tag="woB")
                dmaq[1].dma_start(woB[:], woB_d[:])
                tri = pp.tile([128, 128], f32, tag="tri")
                dmaq[2].dma_start(tri[:], tri_d[:])
                onesf = pp.tile([1, D], f32, tag="onesf")
                nc.vector.memset(onesf[:], 1.0)
                ones = pp.tile([1, D], f32r, tag="ones")
                nc.scalar.copy(ones[:], onesf[:])

                # persistent intermediates: [q0|q1], [k0|k1], [q2], [k2]
                # (projection computes [q2|k2] packed; RoPE splits into two
                # 64-row tiles via cross-partition DVE writes)
                qk_rows = [128, 128, 64, 64]
                qkT = [pp.tile([qk_rows[m], T], f32r, tag=f"qkT{m}",
                               name=f"qkT{m}") for m in range(4)]
                v_sb = pp.tile([128, TQ, HPG, 65], f32r, tag="v_sb")
                onesw = pp.tile([128, TQ * HPG], f32, tag="onesw")
                nc.vector.memset(onesw[:], 1.0)
                nc.scalar.copy(
                    v_sb[:, :, :, 64:65],
                    onesw[:].rearrange("p (a b) -> p a b", b=HPG).rearrange(
                        "p a b -> p a b ()"))
                attnT_A = pp.tile([2 * D, T], f32r, tag="attnTA")
                attnT_B = pp.tile([D, T], f32r, tag="attnTB")
                attn_dst = [(attnT_A, 0), (attnT_A, D), (attnT_B, 0)]

                # ================= QKV phase (scoped pools) =================
                qkv_pool = tc.tile_pool(name=f"qkv{rep}", bufs=1)
                qp = qkv_pool.__enter__()
                qkv_ps_pool = tc.tile_pool(name=f"qkvps{rep}", bufs=8, space="PSUM")
                qps = qkv_ps_pool.__enter__()

                xT = [qp.tile([128, T], f32r, tag=f"xT{c}", name=f"xT{c}")
                      for c in range(NCC)]
                wA = [qp.tile([128, 640], f32r, tag=f"wA{c}", name=f"wA{c}")
                      for c in range(NCC)]
                # weights first (small), then xT column-major in [128, 512]
                # pieces so the first projection chunk's deps arrive in ~2us
                for c in range(NCC):
                    dmaq[c % 3].dma_start(
                        wA[c][:], wA_d[128 * c:128 * (c + 1), :])
                qi = 0
                for n in range(4):
                    for c in range(NCC):
                        nsl = slice(512 * n, 512 * (n + 1))
                        dmaq[qi % 3].dma_start(
                            xT[c][:, nsl], xT_d[128 * c:128 * (c + 1), nsl])
                        qi += 1
                cosT = qp.tile([128, T], f32, tag="cosT")
                sinT = qp.tile([128, T], f32, tag="sinT")
                dmaq[2].dma_start(cosT[:], cosT_d[:])
                dmaq[0].dma_start(sinT[:], sinT_d[:])
                rT = qp.tile([128, 128], f32r, tag="rT")
                dmaq[1].dma_start(rT[:], rT_d[:])

                # q/k projection + RoPE; rot matmuls lag the raw projections
                # by two chunks so PE never stalls on the ACT psum->sbuf copy
                qk_cols = [(0, 128), (128, 256), (256, 384)]
                chunks = [(m, n) for n in range(4) for m in range(3)]
                raws = {}

                def emit_raw(i):
                    m, n = chunks[i]
                    c0, c1 = qk_cols[m]
                    rows = 128
                    tsl = slice(512 * n, 512 * (n + 1))
                    praw = qps.tile([128, 512], f32, tag="ps", name=f"praw{i}")
                    for c in range(NCC):
                        nc.tensor.matmul(
                            praw[0:rows, :], wA[c][:, c0:c1], xT[c][:, tsl],
                            start=(c == 0), stop=(c == NCC - 1))
                    raw = qp.tile([128, 512], f32r, tag="raw", bufs=5,
                                  name=f"raw{i}")
                    nc.scalar.copy(raw[0:rows, :], praw[0:rows, :])
                    raws[i] = raw

                def emit_rope(i):
                    m, n = chunks[i]
                    tsl = slice(512 * n, 512 * (n + 1))
                    raw = raws.pop(i)
                    prot = qps.tile([128, 512], f32, tag="ps", name=f"prot{i}")
                    nc.tensor.matmul(prot[:], rT[:], raw[:], start=True,
                                     stop=True)
                    t1 = qp.tile([128, 512], f32, tag="t1", bufs=3,
                                 name=f"t1_{i}")
                    nc.gpsimd.tensor_tensor(t1[:], raw[:], cosT[:, tsl], mult)
                    t2 = qp.tile([128, 512], f32, tag="t2", bufs=3,
                                 name=f"t2_{i}")
                    nc.vector.tensor_tensor(t2[:], prot[:], sinT[:, tsl], mult)
                    if m < 2:
                        nc.vector.tensor_tensor(qkT[m][:, tsl], t1[:], t2[:],
                                                add)
                    else:
                        # packed [q2|k2]: split to qkT[2]/qkT[3] (cross-part)
                        nc.vector.tensor_tensor(qkT[2][:, tsl], t1[0:64, :],
                                                t2[0:64, :], add)
                        nc.vector.tensor_tensor(qkT[3][:, tsl], t1[64:128, :],
                                                t2[64:128, :], add)

                for i in range(len(chunks)):
                    emit_raw(i)
                    if i >= 2:
                        emit_rope(i - 2)
                for i in (len(chunks) - 2, len(chunks) - 1):
                    emit_rope(i)

                # V projection in natural [t, d] layout
                for t in range(TQ):
                    tsl = slice(128 * t, 128 * (t + 1))
                    pv = qps.tile([128, 256], f32, tag="ps", name=f"pv{t}")
                    for c in range(NCC):
                        nc.tensor.matmul(pv[:], xT[c][:, tsl],
                                         wA[c][:, 384:640], start=(c == 0),
                                         stop=(c == NCC - 1))
                    nc.vector.tensor_copy(
                        v_sb[:, t, :, 0:64],
                        pv[:, 0:192].rearrange("p (h d) -> p h d", d=64))

                qkv_ps_pool.__exit__(None, None, None)
                qkv_pool.__exit__(None, None, None)

                # ========== attention + out projection (interleaved) ==========
                attn_pool = tc.tile_pool(name=f"attn{rep}", bufs=1)
                ap = attn_pool.__enter__()
                attn_ps_pool = tc.tile_pool(name=f"attnps{rep}", bufs=2, space="PSUM")
                aps = attn_ps_pool.__enter__()

                # q/k row views per head: (tile index, partition offset)
                qv = [(0, 0), (0, 64), (2, 0)]
                kv = [(1, 0), (1, 64), (3, 0)]

                for g in range(NGRP):
                    for h in range(HPG):
                        qm, qo = qv[h]
                        km, ko = kv[h]
                        qT = qkT[qm][qo:qo + 64, :]
                        kT = qkT[km][ko:ko + 64, :]
                        nj = (GW // 128) * (g + 1)
                        # pass A: scores + exp (+ causal tri) for every k-chunk
                        pts = []
                        for j in range(nj):
                            dj = j - (GW // 128) * g
                            col0 = 128 * dj if dj >= 0 else 0
                            pscr = aps.tile([128, GW], f32, tag="pscr", bufs=2,
                                            name=f"pscr{g}_{h}_{j}")
                            for s0 in range(col0 - col0 % 512, GW, 512):
                                a0 = max(s0, col0)
                                nc.tensor.matmul(
                                    pscr[:, a0:s0 + 512],
                                    kT[:, 128 * j:128 * (j + 1)],
                                    qT[:, GW * g + a0:GW * g + s0 + 512],
                                    start=True, stop=True)
                            pt = ap.tile([128, GW], f32r, tag="pt", bufs=17,
                                         name=f"pt{g}_{h}_{j}")
                            nc.scalar.activation(pt[:, col0:], pscr[:, col0:],
                                                 Exp, scale=0.125)
                            if dj >= 0:
                                nc.gpsimd.tensor_tensor(
                                    pt[:, col0:col0 + 128],
                                    pt[:, col0:col0 + 128], tri[:], mult)
                            pts.append((pt, col0))
                        # pass B: P^T @ V into two 512-wide accumulators
                        pos = [aps.tile([65, 512], f32, tag="pso", bufs=4,
                                        name=f"po{g}_{h}_{i2}")
                               for i2 in range(GW // 512)]
                        lastw = {}
                        for j in range(nj):
                            _, col0 = pts[j]
                            for s0 in range(col0 - col0 % 512, GW, 512):
                                lastw[s0 // 512] = j
                        for j in range(nj):
                            pt, col0 = pts[j]
                            for s0 in range(col0 - col0 % 512, GW, 512):
                                a0 = max(s0, col0)
                                hv = s0 // 512
                                nc.tensor.matmul(
                                    pos[hv][:, a0 - s0:512], v_sb[:, j, h, :],
                                    pt[:, a0:s0 + 512], start=(j == 0),
                                    stop=(j == lastw[hv]), skip_group_check=True)
                        # normalize per half: attnT = po[0:64] * (1/po[64]),
                        # denominator broadcast across partitions on GPSIMD
                        for hv in range(GW // 512):
                            po = pos[hv]
                            csl = slice(GW * g + 512 * hv, GW * g + 512 * (hv + 1))
                            rc0 = ap.tile([1, 512], f32, tag="rc0", bufs=2,
                                          name=f"rc0{g}_{h}_{hv}")
                            nc.vector.reciprocal(rc0[:], po[64:65, :])
                            pbb = ap.tile([64, 512], f32, tag="pbb", bufs=3,
                                          name=f"pbb{g}_{h}_{hv}")
                            nc.gpsimd.partition_broadcast(pbb[:], rc0[:])
                            dstT, dofs = attn_dst[h]
                            nc.vector.tensor_tensor(dstT[dofs:dofs + D, csl],
                                                    po[0:64, :], pbb[:], mult)

                    # out projection for this g's t-range, from the same pool
                    for t in range((TQ // NGRP) * g, (TQ // NGRP) * (g + 1)):
                        tsl = slice(128 * t, 128 * (t + 1))
                        for c0, cn in ((0, 512), (512, 256)):
                            pout = aps.tile([128, cn], f32, tag="pso", bufs=4,
                                            name=f"pout{t}_{c0}")
                            nc.tensor.matmul(pout[:], attnT_A[:, tsl],
                                             woA[:, c0:c0 + cn], start=True,
                                             stop=False)
                            nc.tensor.matmul(pout[:], attnT_B[:, tsl],
                                             woB[:, c0:c0 + cn], start=False,
                                             stop=True)
                            osb = ap.tile([128, cn], f32, tag=f"osb{c0}", bufs=3,
                                          name=f"osb{t}_{c0}")
                            nc.any.tensor_copy(osb[:], pout[:])
                            dmaq[2 * ((t + (1 if c0 else 0)) % 2)].dma_start(
                                out_d[tsl, c0:c0 + cn], osb[:])

                attn_ps_pool.__exit__(None, None, None)
                attn_pool.__exit__(None, None, None)

    nc.compile()
    return nc


def _host_inputs(x, w_qkv, w_out):
    """Build the 8 per-core input maps."""
    inv_freq = 1.0 / (ROPE_BASE ** (np.arange(0, D, 2, dtype=np.float32) / D))
    t = np.arange(T, dtype=np.float32)
    freqs = t[:, None] * inv_freq[None, :]          # [T, D/2]
    emb = np.concatenate([freqs, freqs], axis=-1)   # [T, D]
    cosT = np.ascontiguousarray(np.cos(emb).T.astype(np.float32))  # [D, T]
    sinT = np.ascontiguousarray(np.sin(emb).T.astype(np.float32))
    cosT2 = np.concatenate([cosT, cosT], axis=0)    # [128, T]
    sinT2 = np.concatenate([sinT, sinT], axis=0)

    # rotate_half permutation as matmul lhsT: rot = R @ q, lhsT = R.T
    R = np.zeros((D, D), np.float32)
    R[0:32, 32:64] = -np.eye(32)
    R[32:64, 0:32] = np.eye(32)
    R2 = np.zeros((128, 128), np.float32)
    R2[0:64, 0:64] = R
    R2[64:128, 64:128] = R
    rT = np.ascontiguousarray(R2.T)

    tri = np.zeros((128, 128), np.float32)
    for kr in range(128):
        tri[kr, kr:] = 1.0

    wq = w_qkv[0:C]
    wk = w_qkv[C:2 * C]
    wv = w_qkv[2 * C:3 * C]

    maps = []
    for core in range(NG):
        b, hg = core // 4, core % 4
        hs = slice(HPG * D * hg, HPG * D * (hg + 1))   # 192 rows of this group
        h2 = HPG * D * hg + 2 * D
        q01 = wq[hs][0:128]                             # [128, C]
        k01 = wk[hs][0:128]
        q2 = wq[h2:h2 + D]
        k2 = wk[h2:h2 + D]
        v3 = wv[hs]                                     # [192, C]
        wA = np.zeros((C, 640), np.float32)
        wA[:, 0:128] = q01.T
        wA[:, 128:256] = k01.T
        wA[:, 256:320] = q2.T
        wA[:, 320:384] = k2.T
        wA[:, 384:576] = v3.T
        wo_h = [w_out[:, HPG * D * hg + D * h: HPG * D * hg + D * (h + 1)].T
                for h in range(HPG)]                    # 3 x [64, C]
        woA = np.concatenate([wo_h[0], wo_h[1]], axis=0)  # [128, C]
        woB = wo_h[2]                                     # [64, C]
        maps.append({
            "xT": np.ascontiguousarray(x[b].T),
            "wA": np.ascontiguousarray(wA),
            "woA": np.ascontiguousarray(woA.astype(np.float32)),
            "woB": np.ascontiguousarray(woB.astype(np.float32)),
            "cosT": cosT2, "sinT": sinT2,
            "rT": rT, "tri": tri,
        })
    return maps


def kernel(x, w_qkv, w_out):
    from concourse.bass_utils import run_bass_kernel_spmd

    if "nc" not in _CACHE:
        _CACHE["nc"] = _build_nc()
    nc = _CACHE["nc"]

    maps = _host_inputs(np.asarray(x, np.float32),
                        np.asarray(w_qkv, np.float32),
                        np.asarray(w_out, np.float32))
    res = run_bass_kernel_spmd(nc, maps, core_ids=list(range(NG))).results
    parts = np.stack([r["out"] for r in res])           # [8, T, C]
    out = np.zeros((B, T, C), np.float32)
    for b in range(B):
        out[b] = parts[4 * b:4 * (b + 1)].sum(axis=0)
    return out



# revision 68
# speedup vs baseline: 1.1767x; 1.1767x over previous
"""Multi-head attention (12 heads, RoPE, causal SDPA) for Trainium2, 8 cores.

Sharding: batch (2) x head-group (4 groups of 3 heads). Each core computes,
for its (batch b, head-group hg): QKV projection for its 3 heads, RoPE,
causal attention, and a partial out-projection [T, C] restricted to its
heads' rows of w_out. The host sums the 4 head-group partials per batch.

Precision: all matmuls in bf16 except the score matmul, which runs in
fp8-e4m3 with DoubleRow perf mode (2x PE throughput, contraction pairs
packed in-partition). Measured end-to-end rel-err ~1e-2 vs fp64 (gate 2e-2).

Layouts (T=2048, C=768, D=64/head, q-groups of 512, k-chunks of 128):
  xbf   [128, 4, 6, 512]  x[b].T bf16: [c-part, t-chunk, c-chunk, t]
  wqk   [128, 6, 3, 128]  qk-proj stationaries (3 psum tiles of 128 rows)
  qf8/kf8 [96, 2, T] fp8  RoPE'd q/k, partition 32h+d (d<32), dim1=(lo,hi)
                          pair layout for DoubleRow scores (contraction 64)
  v_sb  [128, 16, 3, 65]  v in [t, d] layout + ones column (softmax denom)
  pt3   [128, 3, 512]     exp(scores^T) per k-chunk, 3 heads packed (bf16)
  oacc  [128, 3, 65] psum attention out per q-tile, [t, d] layout + denom
  attnT [128, T]          normalized attn out transposed to [d, t] via DMA
                          XBAR block-transposes, feeds out-projection

Scores are computed transposed (S^T[k, q]) so softmax exp lands in [k, q]
which is the PV stationary; PV uses v as the 65-wide moving operand (full
PE utilization). Softmax has no max-subtraction (scores are O(1)); the
denominator is the ones-column of v. Normalization is a per-partition
tensor_scalar in the [t, d] layout.
"""
import numpy as np
import ml_dtypes

B, T, C, H, D = 2, 2048, 768, 12, 64
HPG = 3                    # heads per group
NG = B * (H // HPG)        # 8 cores
ROPE_BASE = 10000.0
TC = 4                     # t-chunks of 512 in qkv phase
NCC = C // 128             # 6 contraction chunks
GW = 512                   # attention q-group width
NGRP = T // GW             # 4 q-groups
NJ = T // 128              # 16 k-chunks

_CACHE = {}
SKIP_QKV = False    # debug: feed qf8/kf8/v_sb from DRAM, attention only
SKIP_ATTN = False   # debug: stop after the QKV+RoPE phase
RAW_ENG = "gpsimd"  # engine for praw psum -> bf16 raw copies
V_ENG = "vector"    # engine for pv psum -> v_sb copies

# psum-tile row maps for the qk projection: per tile i, row r ->
# (kind, head, d). Strips chosen so every RoPE op's partition window is
# legal (windows starting at 32 may not cross 64, etc.):
#   t0 = [qlo(3h, 96) | qhi_h2(32)]
#   t1 = [qhi_h0,h1(64) | klo_h0,h1(64)]
#   t2 = [khi(3h, 96) | klo_h2(32)]


def _row_role(i, r):
    if i == 0:
        return ("q", r // 32, r % 32) if r < 96 else ("q", 2, 32 + r - 96)
    if i == 1:
        if r < 64:
            return ("q", r // 32, 32 + r % 32)
        rr = r - 64
        return ("k", rr // 32, rr % 32)
    if r < 96:
        return ("k", r // 32, 32 + r % 32)
    return ("k", 2, r - 96)


def _build_nc(reps=1):
    from concourse import bacc, tile, mybir

    f32 = mybir.dt.float32
    bf16 = mybir.dt.bfloat16
    f8 = mybir.dt.float8e4
    Exp = mybir.ActivationFunctionType.Exp
    mult = mybir.AluOpType.mult
    add = mybir.AluOpType.add
    sub = mybir.AluOpType.subtract
    DR = mybir.MatmulPerfMode.DoubleRow

    nc = bacc.Bacc("TRN2", target_bir_lowering=False, debug=False,
                   num_devices=NG)

    xbf_d = nc.dram_tensor("xbf", [128, TC, NCC, 512], bf16,
                           kind="ExternalInput").ap()
    wqk_d = nc.dram_tensor("wqk", [128, NCC, 3, 128], bf16,
                           kind="ExternalInput").ap()
    wv_d = nc.dram_tensor("wv", [128, NCC, 192], bf16,
                          kind="ExternalInput").ap()
    woA_d = nc.dram_tensor("woA", [128, C], bf16, kind="ExternalInput").ap()
    woB_d = nc.dram_tensor("woB", [64, C], bf16, kind="ExternalInput").ap()
    cosS_d = nc.dram_tensor("cosS", [128, 3, T], bf16,
                            kind="ExternalInput").ap()
    sinO_d = nc.dram_tensor("sinO", [128, 3, T], bf16,
                            kind="ExternalInput").ap()
    tri3_d = nc.dram_tensor("tri3", [128, HPG, 128], bf16,
                            kind="ExternalInput").ap()
    out_d = nc.dram_tensor("out", [T, C], bf16, kind="ExternalOutput").ap()
    if SKIP_QKV:
        qf8_d = nc.dram_tensor("qf8d", [96, TC, 2, 512], f8,
                               kind="ExternalInput").ap()
        kf8_d = nc.dram_tensor("kf8d", [96, TC, 2, 512], f8,
                               kind="ExternalInput").ap()
        vsb_d = nc.dram_tensor("vsbd", [128, NJ, HPG, 65], bf16,
                               kind="ExternalInput").ap()

    with tile.TileContext(nc) as tc:
      for rep in range(reps):
        # Single SBUF + single PSUM pool per rep: pool boundaries insert
        # hard barriers (a new pool's region waits for the old pool's full
        # drain), which serialized the qkv and attention phases. Tags share
        # rings across phases instead.
        with tc.tile_pool(name=f"pp{rep}", bufs=1) as pp, \
             tc.tile_pool(name=f"wk{rep}", bufs=1) as wk, \
             tc.tile_pool(name=f"ps{rep}", bufs=1, space="PSUM") as ps:
            dq = nc.sync

            # ---- inputs (first c-chunks first: fast lead-in) ----
            wqk = wk.tile([128, NCC, 3, 128], bf16, tag="wqk")
            dq.dma_start(wqk[:, 0:2], wqk_d[:, 0:2])
            xbf = wk.tile([128, TC, NCC, 512], bf16, tag="xbf")
            dq.dma_start(xbf[:, 0, 0:2], xbf_d[:, 0, 0:2])
            dq.dma_start(wqk[:, 2:NCC], wqk_d[:, 2:NCC])
            dq.dma_start(xbf[:, 0, 2:NCC], xbf_d[:, 0, 2:NCC])
            for tcn in range(1, TC):
                dq.dma_start(xbf[:, tcn], xbf_d[:, tcn])
            cosS = wk.tile([128, 3, T], bf16, tag="cosS")
            sinO = wk.tile([128, 3, T], bf16, tag="sinO")
            tcs = slice(0, 512)
            dq.dma_start(cosS[:, :, tcs], cosS_d[:, :, tcs])
            dq.dma_start(sinO[:, :, tcs], sinO_d[:, :, tcs])
            wv = wk.tile([128, NCC, 192], bf16, tag="wv")
            dq.dma_start(wv[:], wv_d[:])
            for tcn in range(1, TC):
                tcs = slice(512 * tcn, 512 * (tcn + 1))
                dq.dma_start(cosS[:, :, tcs], cosS_d[:, :, tcs])
                dq.dma_start(sinO[:, :, tcs], sinO_d[:, :, tcs])
            tri3 = pp.tile([128, HPG, 128], bf16, tag="tri3")
            dq.dma_start(tri3[:], tri3_d[:])
            woA = pp.tile([128, C], bf16, tag="woA")
            dq.dma_start(woA[:], woA_d[:])
            woB = pp.tile([64, C], bf16, tag="woB")
            dq.dma_start(woB[:], woB_d[:])

            qf8 = [pp.tile([96, 2, 512], f8, tag=f"qf8_{i}", name=f"qf8_{i}")
                   for i in range(TC)]
            kf8 = [pp.tile([96, 2, 512], f8, tag=f"kf8_{i}", name=f"kf8_{i}")
                   for i in range(TC)]
            v_sb = pp.tile([128, NJ, HPG, 65], bf16, tag="v_sb")
            onesw = pp.tile([128, NJ * HPG], bf16, tag="onesw")
            nc.vector.memset(onesw[:], 1.0)
            nc.vector.tensor_copy(
                v_sb[:, :, :, 64:65],
                onesw[:].rearrange("p (a b) -> p a b", b=HPG).rearrange(
                    "p a b -> p a b ()"))
            attnT_A = pp.tile([128, T], bf16, tag="attnTA")
            attnT_B = pp.tile([128, T], bf16, tag="attnTB")

            # ================= QKV projection + RoPE =================
            if SKIP_QKV:
                for i in range(TC):
                    dq.dma_start(qf8[i][:], qf8_d[:, i])
                    dq.dma_start(kf8[i][:], kf8_d[:, i])
                dq.dma_start(v_sb[:, :, :, 0:64], vsb_d[:, :, :, 0:64])

            def emit_qkv_chunk(tcn, ring="big"):
                tsl = slice(512 * tcn, 512 * (tcn + 1))
                # chunks 0/1: the 3 projection psum tiles in ONE "big" ring
                # slot (shared tag with attention's pscr — no pool barrier).
                # chunks 2/3 go through the "small" ring so the big ring is
                # purely exp-paced once the attention stream starts.
                if ring == "big":
                    praw3 = ps.tile([128, HPG * 512], f32, tag="big", bufs=2,
                                    name=f"praw3_{tcn}")
                    prs = [praw3[:, 512 * i:512 * (i + 1)] for i in range(3)]
                else:
                    prs = []
                    for i in range(3):
                        pr = ps.tile([128, 512], f32, tag="small", bufs=2,
                                     name=f"praw_{tcn}_{i}")
                        prs.append(pr[:])
                raws = []
                # copies fan out across DVE/ACT (gpsimd cannot read PSUM on
                # hw); ACT helps only before its exp stream ramps up
                copy_eng = (("vector", "scalar", "vector") if tcn < 2 else
                            ("vector", "vector", "vector"))
                for i in range(3):
                    for c in range(NCC):
                        nc.tensor.matmul(prs[i],
                                         wqk[:, c, i, :],
                                         xbf[:, tcn, c, :],
                                         start=(c == 0), stop=(c == NCC - 1))
                    raw = wk.tile([128, 512], bf16, tag=f"raw{i}", bufs=2,
                                  name=f"raw{tcn}_{i}")
                    if copy_eng[i] == "scalar":
                        nc.scalar.copy(raw[:], prs[i])
                    else:
                        getattr(nc, copy_eng[i]).tensor_copy(raw[:], prs[i])
                    raws.append(raw)
                r0, r1, r2 = raws

                At = wk.tile([96, 512], bf16, tag="At", bufs=2)
                Bt = wk.tile([96, 512], bf16, tag="Bt", bufs=2)
                Ct = wk.tile([96, 512], bf16, tag="Ct", bufs=2)
                Dt = wk.tile([96, 512], bf16, tag="Dt", bufs=2)
                # q: lo rows r0[0:96]; hi rows r1[0:64] (h0,1), r0[96:] (h2)
                nc.vector.tensor_tensor(At[:], r0[0:96], cosS[0:96, 0, tsl],
                                        mult)
                nc.vector.tensor_tensor(Bt[0:64], r1[0:64],
                                        sinO[0:64, 1, tsl], mult)
                nc.vector.tensor_tensor(Bt[64:96], r0[96:128],
                                        sinO[96:128, 0, tsl], mult)
                nc.vector.tensor_tensor(Ct[0:64], r1[0:64],
                                        cosS[0:64, 1, tsl], mult)
                nc.vector.tensor_tensor(Ct[64:96], r0[96:128],
                                        cosS[96:128, 0, tsl], mult)
                nc.vector.tensor_tensor(Dt[:], r0[0:96], sinO[0:96, 0, tsl],
                                        mult)
                nc.gpsimd.tensor_tensor(qf8[tcn][:, 0, :], At[:], Bt[:], sub)
                nc.gpsimd.tensor_tensor(qf8[tcn][:, 1, :], Ct[:], Dt[:], add)
                # k: lo rows r1[64:128] (h0,1), r2[96:] (h2); hi r2[0:96]
                At2 = wk.tile([96, 512], bf16, tag="At2", bufs=2)
                Bt2 = wk.tile([96, 512], bf16, tag="Bt2", bufs=2)
                Ct2 = wk.tile([96, 512], bf16, tag="Ct2", bufs=2)
                Dt2 = wk.tile([96, 512], bf16, tag="Dt2", bufs=2)
                nc.vector.tensor_tensor(At2[0:64], r1[64:128],
                                        cosS[64:128, 1, tsl], mult)
                nc.vector.tensor_tensor(At2[64:96], r2[96:128],
                                        cosS[96:128, 2, tsl], mult)
                nc.vector.tensor_tensor(Bt2[:], r2[0:96],
                                        sinO[0:96, 2, tsl], mult)
                nc.vector.tensor_tensor(Ct2[:], r2[0:96],
                                        cosS[0:96, 2, tsl], mult)
                nc.vector.tensor_tensor(Dt2[0:64], r1[64:128],
                                        sinO[64:128, 1, tsl], mult)
                nc.vector.tensor_tensor(Dt2[64:96], r2[96:128],
                                        sinO[96:128, 2, tsl], mult)
                nc.gpsimd.tensor_tensor(kf8[tcn][:, 0, :], At2[:], Bt2[:], sub)
                nc.gpsimd.tensor_tensor(kf8[tcn][:, 1, :], Ct2[:], Dt2[:], add)

                # v projection in natural [t, d] layout
                for st in range(4):
                    t = 4 * tcn + st
                    t128 = slice(128 * st, 128 * (st + 1))
                    pv = ps.tile([128, 512], f32, tag="small", bufs=2,
                                 name=f"pv{t}")
                    for c in range(NCC):
                        nc.tensor.matmul(pv[:, 0:192], xbf[:, tcn, c, t128],
                                        wv[:, c, :], start=(c == 0),
                                        stop=(c == NCC - 1))
                    # chunks 0-1: ACT is idle before the exp stream starts;
                    # later chunks must not steal ACT from exp
                    if tcn < 2:
                        nc.scalar.copy(
                            v_sb[:, t, :, 0:64],
                            pv[:, 0:192].rearrange("p (h d) -> p h d", d=64))
                    else:
                        nc.vector.tensor_copy(
                            v_sb[:, t, :, 0:64],
                            pv[:, 0:192].rearrange("p (h d) -> p h d", d=64))

            # ========== attention (S^T fp8-DR, exp, PV) + out proj ==========
            pts_all = {}

            def emit_score_step(g, j):
                pts = pts_all.setdefault(g, [])
                col0 = max(0, 128 * j - GW * g)
                pscr = ps.tile([128, HPG * 512], f32, tag="big", bufs=2,
                               name=f"pscr{g}_{j}")
                pscr3 = pscr[:].rearrange("p (h w) -> p h w", w=512)
                kc, ko = j // 4, 128 * (j % 4)
                for h in range(HPG):
                    nc.tensor.matmul(
                        pscr3[:, h, col0:512],
                        kf8[kc][32 * h:32 * (h + 1), :, ko:ko + 128],
                        qf8[g][32 * h:32 * (h + 1), :, col0:512],
                        start=True, stop=True, perf_mode=DR)
                pt = wk.tile([128, HPG, 512], bf16, tag="pt", bufs=24,
                             name=f"pt{g}_{j}")
                nc.scalar.activation(pt[:, :, col0:512],
                                     pscr3[:, :, col0:512], Exp,
                                     scale=0.125)
                if j >= 4 * g:
                    nc.vector.tensor_tensor(pt[:, :, col0:col0 + 128],
                                            pt[:, :, col0:col0 + 128],
                                            tri3[:], mult)
                pts.append(pt)

            def emit_scores(g, fillers=(), j0=0):
                # fillers: emit-callbacks (PV / outproj blocks of earlier
                # groups) interleaved between S-steps so PE has work while
                # the 2-deep pscr ring paces S to the ACT exp stream
                fillers = list(fillers)
                for j in range(j0, 4 * (g + 1)):
                    emit_score_step(g, j)
                    if fillers and j >= j0 + 2:
                        fillers.pop(0)()
                for f in fillers:
                    f()

            def emit_pv(g, qt, onA, onB):
                jq = 4 * g + qt
                qsl = slice(128 * qt, 128 * (qt + 1))
                pts = pts_all[g]
                oacc = ps.tile([128, 512], f32, tag="small", bufs=2,
                               name=f"oacc{g}_{qt}")
                oacc3 = oacc[:, 0:HPG * 65].rearrange("p (h d) -> p h d", d=65)
                for h in range(HPG):
                    for j in range(jq + 1):
                        nc.tensor.matmul(
                            oacc3[:, h, :], pts[j][:, h, qsl],
                            v_sb[:, j, h, :], start=(j == 0),
                            stop=(j == jq), skip_group_check=True)
                rden = wk.tile([128, HPG], f32, tag="rden", bufs=2,
                               name=f"rden{g}_{qt}")
                nc.vector.reciprocal(rden[:], oacc3[:, :, 64])
                for h in range(HPG):
                    dst = onA[:, qt, h, :] if h < 2 else onB[:, qt, 0, :]
                    nc.vector.tensor_scalar(dst, oacc3[:, h, 0:64],
                                            rden[:, h:h + 1], None, mult)

            def emit_transpose(g, onA, onB, qts=(0, 4)):
                csl = slice(GW * g + 128 * qts[0], GW * g + 128 * qts[1])
                s = slice(qts[0], qts[1])
                dq.dma_start(
                    attnT_A[:, csl].rearrange("p (a b) -> p a b", b=128),
                    onA[:, s], transpose=True)
                dq.dma_start(
                    attnT_B[:, csl].rearrange("p (a b) -> p a b", b=128),
                    onB[:, s], transpose=True)

            def emit_outproj(g, qts, osb, tags=("small", "small")):
                for qt in qts:
                    tsl = slice(GW * g + 128 * qt, GW * g + 128 * (qt + 1))
                    for c0, cn in ((0, 512), (512, 256)):
                        tag = tags[0 if c0 == 0 else 1]
                        pout = ps.tile(
                            [128, 512] if tag == "small" else [128, HPG * 512],
                            f32, tag=tag, bufs=2, name=f"pout{g}_{qt}_{c0}")
                        nc.tensor.matmul(pout[:, 0:cn], attnT_A[:, tsl],
                                         woA[:, c0:c0 + cn], start=True,
                                         stop=False)
                        nc.tensor.matmul(pout[:, 0:cn],
                                         attnT_B[0:64, tsl],
                                         woB[:, c0:c0 + cn], start=False,
                                         stop=True)
                        if c0 == 0:
                            nc.vector.tensor_copy(osb[:, qt, 0:512],
                                                  pout[:, 0:512])
                        else:
                            nc.vector.tensor_copy(osb[:, qt, 512:768],
                                                  pout[:, 0:256])

            def emit_outdma(g, osb, qts=None):
                if qts is None:
                    gsl, s = slice(GW * g, GW * (g + 1)), slice(0, 4)
                else:
                    gsl = slice(GW * g + 128 * qts[0],
                                GW * g + 128 * (qts[-1] + 1))
                    s = slice(qts[0], qts[-1] + 1)
                dq.dma_start(
                    out_d[gsl, :].rearrange("(a p) c -> p a c", p=128),
                    osb[:, s])

            def mk_onAB(g):
                onA = wk.tile([128, 4, 2, 64], bf16, tag="onA", bufs=2,
                              name=f"onA{g}")
                onB = wk.tile([128, 4, 2, 64], bf16, tag="onB", bufs=2,
                              name=f"onB{g}")
                nc.gpsimd.memset(onB[:, :, 1, :], 0.0)
                return onA, onB

            # Interleaved emission: qkv chunks and g0's score steps share
            # the "big" psum ring, so the exp stream starts as soon as
            # chunk 0's RoPE lands instead of after the whole qkv phase.
            ons = {}
            if SKIP_QKV:
                for g in range(NGRP if not SKIP_ATTN else 0):
                    emit_scores(g) if g == 0 else None
            else:
                # big-ring alloc order tuned so each alloc's 2-back consumer
                # is already done or exp-paced: qkv chunks 2/3 slot between
                # g0's exp-paced score steps
                emit_qkv_chunk(0)
                emit_qkv_chunk(1)
                if not SKIP_ATTN:
                    emit_score_step(0, 0)
                    emit_score_step(0, 1)
                    emit_score_step(0, 2)
                emit_qkv_chunk(2, ring="small")
                if not SKIP_ATTN:
                    emit_score_step(0, 3)
                    emit_score_step(1, 0)
                emit_qkv_chunk(3, ring="small")

            for g in (range(1, NGRP) if not SKIP_ATTN else []):
                fillers = []
                gp = g - 1
                ons[gp] = mk_onAB(gp)

                def mk_pv(gp, qt, last):
                    def f():
                        emit_pv(gp, qt, *ons[gp])
                        if last:
                            emit_transpose(gp, *ons[gp])
                    return f

                # outproj fillers first: they read attnT regions written two
                # groups ago; emitting them before this round's transposes
                # avoids a whole-tile WAR stall on attnT
                if g > 1:
                    go = g - 2
                    osb = wk.tile([128, 4, C], bf16, tag="osb", bufs=2,
                                  name=f"osb{go}")

                    def mk_po(go, qt, osb, last):
                        def f():
                            emit_outproj(go, [qt], osb)
                            if last:
                                emit_outdma(go, osb)
                        return f
                    for qt in range(4):
                        fillers.append(mk_po(go, qt, osb, qt == 3))
                for qt in range(4):
                    fillers.append(mk_pv(gp, qt, qt == 3))
                emit_scores(g, fillers, j0=1 if g == 1 else 0)

            # drain: PV of the last group + the remaining two
            # out-projections, per-qt pipelined
            if SKIP_ATTN:
                osb0 = wk.tile([128, 4, C], bf16, tag="osb", bufs=2)
                nc.vector.tensor_copy(osb0[:, 0, 0:65], v_sb[:, 0, 0, :])
                dq.dma_start(out_d[0:512, :].rearrange(
                    "(a p) c -> p a c", p=128), osb0[:])
            else:
                gl, gp = NGRP - 1, NGRP - 2
                ons[gl] = mk_onAB(gl)
                osbp = wk.tile([128, 4, C], bf16, tag="osb", bufs=2,
                               name=f"osb{gp}")
                osbl = wk.tile([128, 4, C], bf16, tag="osb", bufs=2,
                               name=f"osb{gl}")
                dtag = ("big", "small")
                # all of gp's pouts before any gl transpose writes attnT
                # (whole-tile WAR would stall them otherwise)
                emit_outproj(gp, [0, 1], osbp, tags=dtag)
                emit_pv(gl, 0, *ons[gl])
                emit_outproj(gp, [2], osbp, tags=dtag)
                emit_pv(gl, 1, *ons[gl])
                emit_outproj(gp, [3], osbp, tags=dtag)
                emit_outdma(gp, osbp)
                emit_transpose(gl, *ons[gl], qts=(0, 2))
                emit_pv(gl, 2, *ons[gl])
                emit_pv(gl, 3, *ons[gl])
                emit_outproj(gl, [0, 1], osbl, tags=dtag)
                emit_transpose(gl, *ons[gl], qts=(2, 4))
                emit_outdma(gl, osbl, qts=[0, 1])
                emit_outproj(gl, [2, 3], osbl, tags=dtag)
                emit_outdma(gl, osbl, qts=[2, 3])

    nc.compile()
    return nc


def _host_inputs(x, w_qkv, w_out):
    """Build the 8 per-core input maps."""
    bf = ml_dtypes.bfloat16
    inv_freq = 1.0 / (ROPE_BASE ** (np.arange(0, D, 2, dtype=np.float32) / D))
    t = np.arange(T, dtype=np.float32)
    freqs = t[:, None] * inv_freq[None, :]          # [T, D/2]
    emb = np.concatenate([freqs, freqs], axis=-1)   # [T, D]
    cos = np.cos(emb).astype(np.float32)            # [T, D]
    sin = np.sin(emb).astype(np.float32)

    cosS = np.zeros((128, 3, T), np.float32)
    sinO = np.zeros((128, 3, T), np.float32)
    for i in range(3):
        for r in range(128):
            _, _, d = _row_role(i, r)
            dp = d + 32 if d < 32 else d - 32
            cosS[r, i, :] = cos[:, d]
            sinO[r, i, :] = sin[:, dp]

    tri3 = np.zeros((128, HPG, 128), np.float32)
    for kr in range(128):
        tri3[kr, :, kr:] = 1.0

    maps = []
    for core in range(NG):
        b, hg = core // 4, core % 4
        xT = x[b].T                                     # [C, T]
        xbf = np.ascontiguousarray(
            xT.reshape(NCC, 128, TC, 512).transpose(1, 2, 0, 3)).astype(bf)

        wqk = np.zeros((128, NCC, 3, 128), np.float32)
        for i in range(3):
            for r in range(128):
                kind, h, d = _row_role(i, r)
                row = 64 * (3 * hg + h) + d + (C if kind == "k" else 0)
                wqk[:, :, i, r] = w_qkv[row].reshape(NCC, 128).T
        wv = np.ascontiguousarray(
            w_qkv[2 * C + 192 * hg:2 * C + 192 * (hg + 1)].T.reshape(
                NCC, 128, 192).transpose(1, 0, 2)).astype(bf)
        woA = np.ascontiguousarray(
            w_out[:, 192 * hg:192 * hg + 128].T).astype(bf)
        woB = np.ascontiguousarray(
            w_out[:, 192 * hg + 128:192 * (hg + 1)].T).astype(bf)
        maps.append({
            "xbf": xbf,
            "wqk": wqk.astype(bf),
            "wv": wv,
            "woA": woA, "woB": woB,
            "cosS": cosS.astype(bf), "sinO": sinO.astype(bf),
            "tri3": tri3.astype(bf),
        })
    return maps


def kernel(x, w_qkv, w_out):
    from concourse.bass_utils import run_bass_kernel_spmd

    if "nc" not in _CACHE:
        _CACHE["nc"] = _build_nc()
    nc = _CACHE["nc"]

    maps = _host_inputs(np.asarray(x, np.float32),
                        np.asarray(w_qkv, np.float32),
                        np.asarray(w_out, np.float32))
    res = run_bass_kernel_spmd(nc, maps, core_ids=list(range(NG))).results
    parts = np.stack([np.asarray(r["out"], dtype=np.float32) for r in res])
    out = np.zeros((B, T, C), np.float32)
    for b in range(B):
        out[b] = parts[4 * b:4 * (b + 1)].sum(axis=0)
    return out


# revision 69
# speedup vs baseline: 1.1890x; 1.0104x over previous
"""Multi-head attention (12 heads, RoPE, causal SDPA) for Trainium2, 8 cores.

Sharding: batch (2) x head-group (4 groups of 3 heads). Each core computes,
for its (batch b, head-group hg): QKV projection for its 3 heads, RoPE,
causal attention, and a partial out-projection [T, C] restricted to its
heads' rows of w_out. The host sums the 4 head-group partials per batch.

Precision: all matmuls in bf16 except the score matmul, which runs in
fp8-e4m3 with DoubleRow perf mode (2x PE throughput, contraction pairs
packed in-partition). Measured end-to-end rel-err ~1e-2 vs fp64 (gate 2e-2).

Layouts (T=2048, C=768, D=64/head, q-groups of 512, k-chunks of 128):
  xbf   [128, 4, 6, 512]  x[b].T bf16: [c-part, t-chunk, c-chunk, t]
  wqk   [128, 6, 3, 128]  qk-proj stationaries (3 psum tiles of 128 rows)
  qf8/kf8 [96, 2, T] fp8  RoPE'd q/k, partition 32h+d (d<32), dim1=(lo,hi)
                          pair layout for DoubleRow scores (contraction 64)
  v_sb  [128, 16, 3, 65]  v in [t, d] layout + ones column (softmax denom)
  pt3   [128, 3, 512]     exp(scores^T) per k-chunk, 3 heads packed (bf16)
  oacc  [128, 3, 65] psum attention out per q-tile, [t, d] layout + denom
  attnT [128, T]          normalized attn out transposed to [d, t] via DMA
                          XBAR block-transposes, feeds out-projection

Scores are computed transposed (S^T[k, q]) so softmax exp lands in [k, q]
which is the PV stationary; PV uses v as the 65-wide moving operand (full
PE utilization). Softmax has no max-subtraction (scores are O(1)); the
denominator is the ones-column of v. Normalization is a per-partition
tensor_scalar in the [t, d] layout.
"""
import numpy as np
import ml_dtypes

B, T, C, H, D = 2, 2048, 768, 12, 64
HPG = 3                    # heads per group
NG = B * (H // HPG)        # 8 cores
ROPE_BASE = 10000.0
TC = 4                     # t-chunks of 512 in qkv phase
NCC = C // 128             # 6 contraction chunks
GW = 512                   # attention q-group width
NGRP = T // GW             # 4 q-groups
NJ = T // 128              # 16 k-chunks

_CACHE = {}
SKIP_QKV = False    # debug: feed qf8/kf8/v_sb from DRAM, attention only
SKIP_ATTN = False   # debug: stop after the QKV+RoPE phase
RAW_ENG = "gpsimd"  # engine for praw psum -> bf16 raw copies
V_ENG = "vector"    # engine for pv psum -> v_sb copies

# psum-tile row maps for the qk projection: per tile i, row r ->
# (kind, head, d). Strips chosen so every RoPE op's partition window is
# legal (windows starting at 32 may not cross 64, etc.):
#   t0 = [qlo(3h, 96) | qhi_h2(32)]
#   t1 = [qhi_h0,h1(64) | klo_h0,h1(64)]
#   t2 = [khi(3h, 96) | klo_h2(32)]


def _row_role(i, r):
    if i == 0:
        return ("q", r // 32, r % 32) if r < 96 else ("q", 2, 32 + r - 96)
    if i == 1:
        if r < 64:
            return ("q", r // 32, 32 + r % 32)
        rr = r - 64
        return ("k", rr // 32, rr % 32)
    if r < 96:
        return ("k", r // 32, 32 + r % 32)
    return ("k", 2, r - 96)


def _build_nc(reps=1):
    from concourse import bacc, tile, mybir

    f32 = mybir.dt.float32
    bf16 = mybir.dt.bfloat16
    f8 = mybir.dt.float8e4
    Exp = mybir.ActivationFunctionType.Exp
    mult = mybir.AluOpType.mult
    add = mybir.AluOpType.add
    sub = mybir.AluOpType.subtract
    DR = mybir.MatmulPerfMode.DoubleRow

    nc = bacc.Bacc("TRN2", target_bir_lowering=False, debug=False,
                   num_devices=NG)

    xbf_d = nc.dram_tensor("xbf", [128, TC, NCC, 512], bf16,
                           kind="ExternalInput").ap()
    wqk_d = nc.dram_tensor("wqk", [128, NCC, 3, 128], bf16,
                           kind="ExternalInput").ap()
    wv_d = nc.dram_tensor("wv", [128, NCC, 192], bf16,
                          kind="ExternalInput").ap()
    woA_d = nc.dram_tensor("woA", [128, C], bf16, kind="ExternalInput").ap()
    woB_d = nc.dram_tensor("woB", [64, C], bf16, kind="ExternalInput").ap()
    cosS_d = nc.dram_tensor("cosS", [128, 3, T], bf16,
                            kind="ExternalInput").ap()
    sinO_d = nc.dram_tensor("sinO", [128, 3, T], bf16,
                            kind="ExternalInput").ap()
    tri3_d = nc.dram_tensor("tri3", [128, HPG, 128], bf16,
                            kind="ExternalInput").ap()
    out_d = nc.dram_tensor("out", [T, C], bf16, kind="ExternalOutput").ap()
    if SKIP_QKV:
        qf8_d = nc.dram_tensor("qf8d", [96, TC, 2, 512], f8,
                               kind="ExternalInput").ap()
        kf8_d = nc.dram_tensor("kf8d", [96, TC, 2, 512], f8,
                               kind="ExternalInput").ap()
        vsb_d = nc.dram_tensor("vsbd", [128, NJ, HPG, 65], bf16,
                               kind="ExternalInput").ap()

    with tile.TileContext(nc) as tc:
      for rep in range(reps):
        # Single SBUF + single PSUM pool per rep: pool boundaries insert
        # hard barriers (a new pool's region waits for the old pool's full
        # drain), which serialized the qkv and attention phases. Tags share
        # rings across phases instead.
        with tc.tile_pool(name=f"pp{rep}", bufs=1) as pp, \
             tc.tile_pool(name=f"wk{rep}", bufs=1) as wk, \
             tc.tile_pool(name=f"ps{rep}", bufs=1, space="PSUM") as ps:
            dq = nc.sync

            # ---- inputs (first c-chunks first: fast lead-in) ----
            wqk = wk.tile([128, NCC, 3, 128], bf16, tag="wqk")
            dq.dma_start(wqk[:, 0:2], wqk_d[:, 0:2])
            xbf = wk.tile([128, TC, NCC, 512], bf16, tag="xbf")
            dq.dma_start(xbf[:, 0, 0:2], xbf_d[:, 0, 0:2])
            dq.dma_start(wqk[:, 2:NCC], wqk_d[:, 2:NCC])
            dq.dma_start(xbf[:, 0, 2:NCC], xbf_d[:, 0, 2:NCC])
            for tcn in range(1, TC):
                dq.dma_start(xbf[:, tcn], xbf_d[:, tcn])
            cosS = wk.tile([128, 3, T], bf16, tag="cosS")
            sinO = wk.tile([128, 3, T], bf16, tag="sinO")
            tcs = slice(0, 512)
            dq.dma_start(cosS[:, :, tcs], cosS_d[:, :, tcs])
            dq.dma_start(sinO[:, :, tcs], sinO_d[:, :, tcs])
            wv = wk.tile([128, NCC, 192], bf16, tag="wv")
            dq.dma_start(wv[:], wv_d[:])
            for tcn in range(1, TC):
                tcs = slice(512 * tcn, 512 * (tcn + 1))
                dq.dma_start(cosS[:, :, tcs], cosS_d[:, :, tcs])
                dq.dma_start(sinO[:, :, tcs], sinO_d[:, :, tcs])
            tri3 = pp.tile([128, HPG, 128], bf16, tag="tri3")
            dq.dma_start(tri3[:], tri3_d[:])
            woA = pp.tile([128, C], bf16, tag="woA")
            dq.dma_start(woA[:], woA_d[:])
            woB = pp.tile([64, C], bf16, tag="woB")
            dq.dma_start(woB[:], woB_d[:])

            qf8 = [pp.tile([96, 2, 512], f8, tag=f"qf8_{i}", name=f"qf8_{i}")
                   for i in range(TC)]
            kf8 = [pp.tile([96, 2, 512], f8, tag=f"kf8_{i}", name=f"kf8_{i}")
                   for i in range(TC)]
            v_sb = pp.tile([128, NJ, HPG, 65], bf16, tag="v_sb")
            onesw = pp.tile([128, NJ * HPG], bf16, tag="onesw")
            nc.vector.memset(onesw[:], 1.0)
            nc.vector.tensor_copy(
                v_sb[:, :, :, 64:65],
                onesw[:].rearrange("p (a b) -> p a b", b=HPG).rearrange(
                    "p a b -> p a b ()"))
            attnT_A = pp.tile([128, T], bf16, tag="attnTA")
            attnT_B = pp.tile([128, T], bf16, tag="attnTB")

            # ================= QKV projection + RoPE =================
            if SKIP_QKV:
                for i in range(TC):
                    dq.dma_start(qf8[i][:], qf8_d[:, i])
                    dq.dma_start(kf8[i][:], kf8_d[:, i])
                dq.dma_start(v_sb[:, :, :, 0:64], vsb_d[:, :, :, 0:64])

            def emit_qkv_chunk(tcn, ring="big"):
                tsl = slice(512 * tcn, 512 * (tcn + 1))
                # chunks 0/1: the 3 projection psum tiles in ONE "big" ring
                # slot (shared tag with attention's pscr — no pool barrier).
                # chunks 2/3 go through the "small" ring so the big ring is
                # purely exp-paced once the attention stream starts.
                if ring == "big":
                    praw3 = ps.tile([128, HPG * 512], f32, tag="big", bufs=2,
                                    name=f"praw3_{tcn}")
                    prs = [praw3[:, 512 * i:512 * (i + 1)] for i in range(3)]
                else:
                    prs = []
                    for i in range(3):
                        pr = ps.tile([128, 512], f32, tag="small", bufs=2,
                                     name=f"praw_{tcn}_{i}")
                        prs.append(pr[:])
                raws = []
                # copies fan out across DVE/ACT (gpsimd cannot read PSUM on
                # hw); ACT helps only before its exp stream ramps up
                copy_eng = (("vector", "scalar", "vector") if tcn < 2 else
                            ("vector", "vector", "vector"))
                for i in range(3):
                    for c in range(NCC):
                        nc.tensor.matmul(prs[i],
                                         wqk[:, c, i, :],
                                         xbf[:, tcn, c, :],
                                         start=(c == 0), stop=(c == NCC - 1))
                    raw = wk.tile([128, 512], bf16, tag=f"raw{i}", bufs=2,
                                  name=f"raw{tcn}_{i}")
                    if copy_eng[i] == "scalar":
                        nc.scalar.copy(raw[:], prs[i])
                    else:
                        getattr(nc, copy_eng[i]).tensor_copy(raw[:], prs[i])
                    raws.append(raw)
                r0, r1, r2 = raws

                At = wk.tile([96, 512], bf16, tag="At", bufs=2)
                Bt = wk.tile([96, 512], bf16, tag="Bt", bufs=2)
                Ct = wk.tile([96, 512], bf16, tag="Ct", bufs=2)
                Dt = wk.tile([96, 512], bf16, tag="Dt", bufs=2)
                # q: lo rows r0[0:96]; hi rows r1[0:64] (h0,1), r0[96:] (h2)
                nc.vector.tensor_tensor(At[:], r0[0:96], cosS[0:96, 0, tsl],
                                        mult)
                nc.vector.tensor_tensor(Bt[0:64], r1[0:64],
                                        sinO[0:64, 1, tsl], mult)
                nc.vector.tensor_tensor(Bt[64:96], r0[96:128],
                                        sinO[96:128, 0, tsl], mult)
                nc.vector.tensor_tensor(Ct[0:64], r1[0:64],
                                        cosS[0:64, 1, tsl], mult)
                nc.vector.tensor_tensor(Ct[64:96], r0[96:128],
                                        cosS[96:128, 0, tsl], mult)
                nc.vector.tensor_tensor(Dt[:], r0[0:96], sinO[0:96, 0, tsl],
                                        mult)
                nc.gpsimd.tensor_tensor(qf8[tcn][:, 0, :], At[:], Bt[:], sub)
                nc.gpsimd.tensor_tensor(qf8[tcn][:, 1, :], Ct[:], Dt[:], add)
                # k: lo rows r1[64:128] (h0,1), r2[96:] (h2); hi r2[0:96]
                At2 = wk.tile([96, 512], bf16, tag="At2", bufs=2)
                Bt2 = wk.tile([96, 512], bf16, tag="Bt2", bufs=2)
                Ct2 = wk.tile([96, 512], bf16, tag="Ct2", bufs=2)
                Dt2 = wk.tile([96, 512], bf16, tag="Dt2", bufs=2)
                nc.vector.tensor_tensor(At2[0:64], r1[64:128],
                                        cosS[64:128, 1, tsl], mult)
                nc.vector.tensor_tensor(At2[64:96], r2[96:128],
                                        cosS[96:128, 2, tsl], mult)
                nc.vector.tensor_tensor(Bt2[:], r2[0:96],
                                        sinO[0:96, 2, tsl], mult)
                nc.vector.tensor_tensor(Ct2[:], r2[0:96],
                                        cosS[0:96, 2, tsl], mult)
                nc.vector.tensor_tensor(Dt2[0:64], r1[64:128],
                                        sinO[64:128, 1, tsl], mult)
                nc.vector.tensor_tensor(Dt2[64:96], r2[96:128],
                                        sinO[96:128, 2, tsl], mult)
                nc.gpsimd.tensor_tensor(kf8[tcn][:, 0, :], At2[:], Bt2[:], sub)
                nc.gpsimd.tensor_tensor(kf8[tcn][:, 1, :], Ct2[:], Dt2[:], add)

                # v projection in natural [t, d] layout
                for st in range(4):
                    t = 4 * tcn + st
                    t128 = slice(128 * st, 128 * (st + 1))
                    pv = ps.tile([128, 512], f32, tag="small", bufs=2,
                                 name=f"pv{t}")
                    for c in range(NCC):
                        nc.tensor.matmul(pv[:, 0:192], xbf[:, tcn, c, t128],
                                        wv[:, c, :], start=(c == 0),
                                        stop=(c == NCC - 1))
                    # chunks 0-1: ACT is idle before the exp stream starts;
                    # later chunks must not steal ACT from exp
                    if tcn < 2:
                        nc.scalar.copy(
                            v_sb[:, t, :, 0:64],
                            pv[:, 0:192].rearrange("p (h d) -> p h d", d=64))
                    else:
                        nc.vector.tensor_copy(
                            v_sb[:, t, :, 0:64],
                            pv[:, 0:192].rearrange("p (h d) -> p h d", d=64))

            # ========== attention (S^T fp8-DR, exp, PV) + out proj ==========
            pts_all = {}

            def emit_score_step(g, j):
                pts = pts_all.setdefault(g, [])
                col0 = max(0, 128 * j - GW * g)
                pscr = ps.tile([128, HPG * 512], f32, tag="big", bufs=2,
                               name=f"pscr{g}_{j}")
                pscr3 = pscr[:].rearrange("p (h w) -> p h w", w=512)
                kc, ko = j // 4, 128 * (j % 4)
                for h in range(HPG):
                    nc.tensor.matmul(
                        pscr3[:, h, col0:512],
                        kf8[kc][32 * h:32 * (h + 1), :, ko:ko + 128],
                        qf8[g][32 * h:32 * (h + 1), :, col0:512],
                        start=True, stop=True, perf_mode=DR)
                pt = wk.tile([128, HPG, 512], bf16, tag="pt", bufs=24,
                             name=f"pt{g}_{j}")
                nc.scalar.activation(pt[:, :, col0:512],
                                     pscr3[:, :, col0:512], Exp,
                                     scale=0.125)
                if j >= 4 * g:
                    nc.gpsimd.tensor_tensor(pt[:, :, col0:col0 + 128],
                                            pt[:, :, col0:col0 + 128],
                                            tri3[:], mult)
                pts.append(pt)

            def emit_scores(g, fillers=(), j0=0):
                # fillers: emit-callbacks (PV / outproj blocks of earlier
                # groups) interleaved between S-steps so PE has work while
                # the 2-deep pscr ring paces S to the ACT exp stream
                fillers = list(fillers)
                for j in range(j0, 4 * (g + 1)):
                    emit_score_step(g, j)
                    if fillers and j >= j0 + 2:
                        fillers.pop(0)()
                for f in fillers:
                    f()

            def emit_pv(g, qt, onA, onB):
                jq = 4 * g + qt
                qsl = slice(128 * qt, 128 * (qt + 1))
                pts = pts_all[g]
                oacc = ps.tile([128, 512], f32, tag="small", bufs=2,
                               name=f"oacc{g}_{qt}")
                oacc3 = oacc[:, 0:HPG * 65].rearrange("p (h d) -> p h d", d=65)
                for h in range(HPG):
                    for j in range(jq + 1):
                        nc.tensor.matmul(
                            oacc3[:, h, :], pts[j][:, h, qsl],
                            v_sb[:, j, h, :], start=(j == 0),
                            stop=(j == jq), skip_group_check=True)
                rden = wk.tile([128, HPG], f32, tag="rden", bufs=2,
                               name=f"rden{g}_{qt}")
                nc.vector.reciprocal(rden[:], oacc3[:, :, 64])
                for h in range(HPG):
                    dst = onA[:, qt, h, :] if h < 2 else onB[:, qt, 0, :]
                    nc.vector.tensor_scalar(dst, oacc3[:, h, 0:64],
                                            rden[:, h:h + 1], None, mult)

            def emit_transpose(g, onA, onB, qts=(0, 4)):
                csl = slice(GW * g + 128 * qts[0], GW * g + 128 * qts[1])
                s = slice(qts[0], qts[1])
                dq.dma_start(
                    attnT_A[:, csl].rearrange("p (a b) -> p a b", b=128),
                    onA[:, s], transpose=True)
                dq.dma_start(
                    attnT_B[:, csl].rearrange("p (a b) -> p a b", b=128),
                    onB[:, s], transpose=True)

            def emit_outproj(g, qts, osb, tags=("small", "small")):
                for qt in qts:
                    tsl = slice(GW * g + 128 * qt, GW * g + 128 * (qt + 1))
                    for c0, cn in ((0, 512), (512, 256)):
                        tag = tags[0 if c0 == 0 else 1]
                        pout = ps.tile(
                            [128, 512] if tag == "small" else [128, HPG * 512],
                            f32, tag=tag, bufs=2, name=f"pout{g}_{qt}_{c0}")
                        nc.tensor.matmul(pout[:, 0:cn], attnT_A[:, tsl],
                                         woA[:, c0:c0 + cn], start=True,
                                         stop=False)
                        nc.tensor.matmul(pout[:, 0:cn],
                                         attnT_B[0:64, tsl],
                                         woB[:, c0:c0 + cn], start=False,
                                         stop=True)
                        if c0 == 0:
                            nc.vector.tensor_copy(osb[:, qt, 0:512],
                                                  pout[:, 0:512])
                        else:
                            nc.vector.tensor_copy(osb[:, qt, 512:768],
                                                  pout[:, 0:256])

            def emit_outdma(g, osb, qts=None):
                if qts is None:
                    gsl, s = slice(GW * g, GW * (g + 1)), slice(0, 4)
                else:
                    gsl = slice(GW * g + 128 * qts[0],
                                GW * g + 128 * (qts[-1] + 1))
                    s = slice(qts[0], qts[-1] + 1)
                dq.dma_start(
                    out_d[gsl, :].rearrange("(a p) c -> p a c", p=128),
                    osb[:, s])

            def mk_onAB(g):
                onA = wk.tile([128, 4, 2, 64], bf16, tag="onA", bufs=2,
                              name=f"onA{g}")
                onB = wk.tile([128, 4, 2, 64], bf16, tag="onB", bufs=2,
                              name=f"onB{g}")
                nc.gpsimd.memset(onB[:, :, 1, :], 0.0)
                return onA, onB

            # Interleaved emission: qkv chunks and g0's score steps share
            # the "big" psum ring, so the exp stream starts as soon as
            # chunk 0's RoPE lands instead of after the whole qkv phase.
            ons = {}
            if SKIP_QKV:
                for g in range(NGRP if not SKIP_ATTN else 0):
                    emit_scores(g) if g == 0 else None
            else:
                # big-ring alloc order tuned so each alloc's 2-back consumer
                # is already done or exp-paced: qkv chunks 2/3 slot between
                # g0's exp-paced score steps
                emit_qkv_chunk(0)
                emit_qkv_chunk(1)
                if not SKIP_ATTN:
                    emit_score_step(0, 0)
                    emit_score_step(0, 1)
                    emit_score_step(0, 2)
                emit_qkv_chunk(2, ring="small")
                if not SKIP_ATTN:
                    emit_score_step(0, 3)
                    emit_score_step(1, 0)
                emit_qkv_chunk(3, ring="small")

            for g in (range(1, NGRP) if not SKIP_ATTN else []):
                fillers = []
                gp = g - 1
                ons[gp] = mk_onAB(gp)

                def mk_pv(gp, qt, last):
                    def f():
                        emit_pv(gp, qt, *ons[gp])
                        if last:
                            emit_transpose(gp, *ons[gp])
                    return f

                # outproj fillers first: they read attnT regions written two
                # groups ago; emitting them before this round's transposes
                # avoids a whole-tile WAR stall on attnT
                if g > 1:
                    go = g - 2
                    osb = wk.tile([128, 4, C], bf16, tag="osb", bufs=2,
                                  name=f"osb{go}")

                    def mk_po(go, qt, osb, last):
                        def f():
                            emit_outproj(go, [qt], osb)
                            if last:
                                emit_outdma(go, osb)
                        return f
                    for qt in range(4):
                        fillers.append(mk_po(go, qt, osb, qt == 3))
                for qt in range(4):
                    fillers.append(mk_pv(gp, qt, qt == 3))
                emit_scores(g, fillers, j0=1 if g == 1 else 0)

            # drain: PV of the last group + the remaining two
            # out-projections, per-qt pipelined
            if SKIP_ATTN:
                osb0 = wk.tile([128, 4, C], bf16, tag="osb", bufs=2)
                nc.vector.tensor_copy(osb0[:, 0, 0:65], v_sb[:, 0, 0, :])
                dq.dma_start(out_d[0:512, :].rearrange(
                    "(a p) c -> p a c", p=128), osb0[:])
            else:
                gl, gp = NGRP - 1, NGRP - 2
                ons[gl] = mk_onAB(gl)
                osbp = wk.tile([128, 4, C], bf16, tag="osb", bufs=2,
                               name=f"osb{gp}")
                osbl = wk.tile([128, 4, C], bf16, tag="osb", bufs=2,
                               name=f"osb{gl}")
                dtag = ("big", "small")
                # all of gp's pouts before any gl transpose writes attnT
                # (whole-tile WAR would stall them otherwise)
                emit_outproj(gp, [0, 1], osbp, tags=dtag)
                emit_pv(gl, 0, *ons[gl])
                emit_outproj(gp, [2], osbp, tags=dtag)
                emit_pv(gl, 1, *ons[gl])
                emit_outproj(gp, [3], osbp, tags=dtag)
                emit_outdma(gp, osbp)
                emit_transpose(gl, *ons[gl], qts=(0, 2))
                emit_pv(gl, 2, *ons[gl])
                emit_pv(gl, 3, *ons[gl])
                emit_outproj(gl, [0, 1], osbl, tags=dtag)
                emit_transpose(gl, *ons[gl], qts=(2, 4))
                emit_outdma(gl, osbl, qts=[0, 1])
                emit_outproj(gl, [2, 3], osbl, tags=dtag)
                emit_outdma(gl, osbl, qts=[2, 3])

    nc.compile()
    return nc


def _host_inputs(x, w_qkv, w_out):
    """Build the 8 per-core input maps."""
    bf = ml_dtypes.bfloat16
    inv_freq = 1.0 / (ROPE_BASE ** (np.arange(0, D, 2, dtype=np.float32) / D))
    t = np.arange(T, dtype=np.float32)
    freqs = t[:, None] * inv_freq[None, :]          # [T, D/2]
    emb = np.concatenate([freqs, freqs], axis=-1)   # [T, D]
    cos = np.cos(emb).astype(np.float32)            # [T, D]
    sin = np.sin(emb).astype(np.float32)

    cosS = np.zeros((128, 3, T), np.float32)
    sinO = np.zeros((128, 3, T), np.float32)
    for i in range(3):
        for r in range(128):
            _, _, d = _row_role(i, r)
            dp = d + 32 if d < 32 else d - 32
            cosS[r, i, :] = cos[:, d]
            sinO[r, i, :] = sin[:, dp]

    tri3 = np.zeros((128, HPG, 128), np.float32)
    for kr in range(128):
        tri3[kr, :, kr:] = 1.0

    maps = []
    for core in range(NG):
        b, hg = core // 4, core % 4
        xT = x[b].T                                     # [C, T]
        xbf = np.ascontiguousarray(
            xT.reshape(NCC, 128, TC, 512).transpose(1, 2, 0, 3)).astype(bf)

        wqk = np.zeros((128, NCC, 3, 128), np.float32)
        for i in range(3):
            for r in range(128):
                kind, h, d = _row_role(i, r)
                row = 64 * (3 * hg + h) + d + (C if kind == "k" else 0)
                wqk[:, :, i, r] = w_qkv[row].reshape(NCC, 128).T
        wv = np.ascontiguousarray(
            w_qkv[2 * C + 192 * hg:2 * C + 192 * (hg + 1)].T.reshape(
                NCC, 128, 192).transpose(1, 0, 2)).astype(bf)
        woA = np.ascontiguousarray(
            w_out[:, 192 * hg:192 * hg + 128].T).astype(bf)
        woB = np.ascontiguousarray(
            w_out[:, 192 * hg + 128:192 * (hg + 1)].T).astype(bf)
        maps.append({
            "xbf": xbf,
            "wqk": wqk.astype(bf),
            "wv": wv,
            "woA": woA, "woB": woB,
            "cosS": cosS.astype(bf), "sinO": sinO.astype(bf),
            "tri3": tri3.astype(bf),
        })
    return maps


def kernel(x, w_qkv, w_out):
    from concourse.bass_utils import run_bass_kernel_spmd

    if "nc" not in _CACHE:
        _CACHE["nc"] = _build_nc()
    nc = _CACHE["nc"]

    maps = _host_inputs(np.asarray(x, np.float32),
                        np.asarray(w_qkv, np.float32),
                        np.asarray(w_out, np.float32))
    res = run_bass_kernel_spmd(nc, maps, core_ids=list(range(NG))).results
    parts = np.stack([np.asarray(r["out"], dtype=np.float32) for r in res])
    out = np.zeros((B, T, C), np.float32)
    for b in range(B):
        out[b] = parts[4 * b:4 * (b + 1)].sum(axis=0)
    return out
